# revision 11
# baseline (speedup 1.0000x reference)
"""Causal self-attention with scratch-memory side channel — Trainium2 Bass kernel.

Problem: nn_CausalSelfAttention (B=4, T=1024, HD=1024, NH=16, DH=64, DS=64).

Sharding: 8 cores = 4 batches x 2 head-groups (8 heads each). Each core:
  - computes qkv projections for its heads (tensor-parallel on heads),
  - scratch projections Qs/Ks/Vs for its heads,
  - transposed score matrices S^T[k,q] = (Kc^T)^T @ Qc^T (+ scratch term
    accumulated in PSUM), exp'd straight into bf16 P^T tiles,
  - causal mask via affine_select on the diagonal chunks only,
  - P@[Vc|Vs|ones] giving content, scratch-out and softmax denominators in
    one PSUM accumulation,
  - row stats (sum/sumsq of unmasked logits per query row, needed for the
    delta-rms / row-std-ratio outputs) via the quadratic-form trick:
    sum_k (q.k)^2 = q^T (sum_k k k^T) q, so a [64,65] G-matmul per q-tile
    replaces full passes over the 1024x1024 score matrices,
  - the output-projection partial for its 512 content dims.
Host glue: shards inputs, sums the two head-group partials per batch, adds
out_b, assembles scratch_out, and computes the two scalar outputs from the
per-row stats in float64.

Matmul dtypes: float32r (full-rate TF32-ish) for every N>=512 matmul,
bf16 for P@V (P is exp output; accumulation stays fp32 in PSUM).
"""

import numpy as np

import concourse.bass as bass
import concourse.mybir as mybir
from concourse import bacc
from concourse.tile import TileContext
from concourse.bass_utils import run_bass_kernel_spmd
from concourse.masks import make_identity

F32 = mybir.dt.float32
F32R = mybir.dt.float32r
BF16 = mybir.dt.bfloat16

B, T, HD, NH = 4, 1024, 1024, 16
DH = HD // NH          # 64
DS = 64
NHL = NH // 2          # 8 local heads per core
GD = NHL * DH          # 512 local content dims
N_CORES = 8
NT = T // 128          # 8 t-tiles
NKC = T // 512         # 2 moving chunks

# stash for test harness
LAST = {}

_prog_cache = {}


def _build_program(add_s: bool, add_bias: bool):
    """Build the SPMD Bass program (same for every core; data differs)."""
    nc = bacc.Bacc(None, target_bir_lowering=False)

    dataT = nc.dram_tensor("data_t", [HD, T], F32, kind="ExternalInput")
    wqk = nc.dram_tensor("wqk_t", [HD, 2 * GD], F32, kind="ExternalInput")
    wv = nc.dram_tensor("wv_t", [HD, GD], F32, kind="ExternalInput")
    sct = nc.dram_tensor("scratch_t", [NHL, DS, T], F32, kind="ExternalInput")
    wqs = nc.dram_tensor("wqs", [NHL, DS, DS], F32, kind="ExternalInput")
    wks = nc.dram_tensor("wks", [NHL, DS, DS], F32, kind="ExternalInput")
    wvs = nc.dram_tensor("wvs", [NHL, DS, DS], F32, kind="ExternalInput")
    wo = nc.dram_tensor("wo_t", [GD, HD], F32, kind="ExternalInput")
    if add_bias:
        bqk = nc.dram_tensor("bqk", [1, 2 * GD], F32, kind="ExternalInput")
        bv = nc.dram_tensor("bv", [1, GD], F32, kind="ExternalInput")

    outp = nc.dram_tensor("partial", [T, HD], F32, kind="ExternalOutput")
    scro = nc.dram_tensor("scr_out", [NHL, T, DS], F32, kind="ExternalOutput")
    stats = nc.dram_tensor("stats", [NT, 128, 4 * NHL], F32, kind="ExternalOutput")

    with TileContext(nc) as tc:
        with tc.tile_pool(name="persist", bufs=1) as pp, \
             tc.tile_pool(name="const", bufs=1) as pc:
            ident = pc.tile([128, 128], F32)
            make_identity(nc, ident[:])

            # persistent tiles
            qkT = pp.tile([128, NT, T], F32R)        # Q^T|K^T rows (qk dims), 32KB/p
            qnat = pp.tile([128, NT, GD], F32)      # Q natural [t, qdims]
            qsT2 = pp.tile([128, 4, T], F32R)        # Qs^T head-parity packed
            ksT2 = pp.tile([128, 4, T], F32R)
            qsnat = pp.tile([128, NT, GD], F32)     # Qs natural
            vall = pp.tile([128, NT, NHL, 132], BF16)  # [Vc|Vs|ones] per (ktile, head)
            content = pp.tile([128, NT, GD], F32)
            stats_sb = pp.tile([128, NT, 4 * NHL], F32)
            gc_rhs = pp.tile([128, 4, 66], F32R)     # [M_Kc | kcol_c] per head pair
            gs_rhs = pp.tile([128, 4, 66], F32R)
            nc.vector.memset(gc_rhs[:, :, 65:66].bitcast(mybir.dt.uint32), 0)
            nc.vector.memset(gs_rhs[:, :, 65:66].bitcast(mybir.dt.uint32), 0)

            # ---------------- Phase A: content projections ----------------
            with tc.tile_pool(name="phA_d", bufs=1) as pAd, \
                 tc.tile_pool(name="phA_ps", bufs=2, space="PSUM") as pAp:
                dT = pAd.tile([128, NT, T], F32R)
                for m in range(NT):
                    nc.sync.dma_start(dT[:, m, :],
                                      dataT[128 * m:128 * (m + 1), :].bitcast(F32R))
                if add_bias:
                    ones_row = pAd.tile([1, T], F32R)
                    nc.vector.memset(ones_row[:], 1.0)
                    bqk_sb = pAd.tile([1, 2 * GD], F32R)
                    nc.sync.dma_start(bqk_sb[:], bqk[:].bitcast(F32R))

                # --- Q-side weights: qkT m-tiles 0..3 + Q natural ---
                with tc.tile_pool(name="phA_w1", bufs=1) as pAw1:
                    wq_t = pAw1.tile([128, NT, GD], F32R)
                    for m in range(NT):
                        nc.sync.dma_start(
                            wq_t[:, m, :], wqk[128 * m:128 * (m + 1), 0:GD].bitcast(F32R))
                    for m in range(4):
                        ps = pAp.tile([128, T], F32, tag="projps")
                        for n in range(NKC):
                            for k in range(NT):
                                nc.tensor.matmul(
                                    ps[:, 512 * n:512 * (n + 1)],
                                    wq_t[:, k, 128 * m:128 * (m + 1)],
                                    dT[:, k, 512 * n:512 * (n + 1)],
                                    start=(k == 0),
                                    stop=(k == NT - 1 and not add_bias))
                            if add_bias:
                                nc.tensor.matmul(
                                    ps[:, 512 * n:512 * (n + 1)],
                                    bqk_sb[:, 128 * m:128 * (m + 1)],
                                    ones_row[:, 512 * n:512 * (n + 1)],
                                    start=False, stop=True)
                        nc.any.tensor_copy(qkT[:, m, :], ps[:])
                    for m in range(NT):
                        ps = pAp.tile([128, T], F32, tag="projps", name="ps_qn")
                        for k in range(NT):
                            nc.tensor.matmul(
                                ps[:, 0:GD], dT[:, k, 128 * m:128 * (m + 1)],
                                wq_t[:, k, :],
                                start=(k == 0), stop=(k == NT - 1 and not add_bias))
                        if add_bias:
                            nc.tensor.matmul(
                                ps[:, 0:GD], ones_row[:, 128 * m:128 * (m + 1)],
                                bqk_sb[:, 0:GD], start=False, stop=True)
                        nc.any.tensor_copy(qnat[:, m, :], ps[:, 0:GD])

                # --- K-side weights: qkT m-tiles 4..7 + K natural + M_Kc ---
                with tc.tile_pool(name="phA_kn", bufs=1) as pAk:
                    knat = pAk.tile([128, NT, GD], F32R)
                    with tc.tile_pool(name="phA_w2", bufs=1) as pAw2:
                        wk_t = pAw2.tile([128, NT, GD], F32R)
                        for m in range(NT):
                            nc.sync.dma_start(
                                wk_t[:, m, :],
                                wqk[128 * m:128 * (m + 1), GD:2 * GD].bitcast(F32R))
                        for m in range(4):
                            ps = pAp.tile([128, T], F32, tag="projps", name="ps_kT")
                            for n in range(NKC):
                                for k in range(NT):
                                    nc.tensor.matmul(
                                        ps[:, 512 * n:512 * (n + 1)],
                                        wk_t[:, k, 128 * m:128 * (m + 1)],
                                        dT[:, k, 512 * n:512 * (n + 1)],
                                        start=(k == 0),
                                        stop=(k == NT - 1 and not add_bias))
                                if add_bias:
                                    nc.tensor.matmul(
                                        ps[:, 512 * n:512 * (n + 1)],
                                        bqk_sb[:, GD + 128 * m:GD + 128 * (m + 1)],
                                        ones_row[:, 512 * n:512 * (n + 1)],
                                        start=False, stop=True)
                            nc.any.tensor_copy(qkT[:, 4 + m, :], ps[:])
                        for m in range(NT):
                            ps = pAp.tile([128, T], F32, tag="projps", name="ps_kn")
                            for k in range(NT):
                                nc.tensor.matmul(
                                    ps[:, 0:GD], dT[:, k, 128 * m:128 * (m + 1)],
                                    wk_t[:, k, :],
                                    start=(k == 0),
                                    stop=(k == NT - 1 and not add_bias))
                            if add_bias:
                                nc.tensor.matmul(
                                    ps[:, 0:GD], ones_row[:, 128 * m:128 * (m + 1)],
                                    bqk_sb[:, GD:2 * GD], start=False, stop=True)
                            nc.any.tensor_copy(knat[:, m, :], ps[:, 0:GD])

                    # M_Kc per head (accumulate over m in a base-0 psum tile)
                    for h in range(NHL):
                        mk = pAp.tile([128, 64], F32, tag="mkps", name="mk_c")
                        for m in range(NT):
                            nc.tensor.matmul(
                                mk[0:64, :],
                                knat[:, m, 64 * h:64 * h + 64],
                                knat[:, m, 64 * h:64 * h + 64],
                                start=(m == 0), stop=(m == NT - 1))
                        if h % 2 == 0:
                            nc.any.tensor_copy(gc_rhs[0:64, h // 2, 0:64], mk[0:64, :])
                        else:
                            mk_sb = pAk.tile([64, 64], F32R, tag="mk_sb", bufs=2)
                            nc.any.tensor_copy(mk_sb[:], mk[0:64, :])
                            nc.sync.dma_start(gc_rhs[64:128, h // 2, 0:64], mk_sb[:])

                    # kcol_c per head -> gc_rhs[., p, 64:65]
                    with nc.allow_low_precision("f32r kcol sums, eps ~1e-3 ok"):
                        for h in range(NHL):
                            pb = (h % 2) * 64
                            nc.vector.tensor_reduce(
                                out=gc_rhs[pb:pb + 64, h // 2, 64:65],
                                in_=qkT[pb:pb + 64, 4 + h // 2, :],
                                axis=mybir.AxisListType.X, op=mybir.AluOpType.add)

                # V natural while dT is alive
                with tc.tile_pool(name="phA_v", bufs=1) as pAv:
                    wvt = pAv.tile([128, NT, GD], F32R)
                    for m in range(NT):
                        nc.sync.dma_start(wvt[:, m, :],
                                          wv[128 * m:128 * (m + 1), :].bitcast(F32R))
                    if add_bias:
                        bv_sb = pAv.tile([1, GD], F32R)
                        nc.sync.dma_start(bv_sb[:], bv[:].bitcast(F32R))
                    for m in range(NT):
                        ps = pAp.tile([128, T], F32, tag="projps", name="ps_v")
                        for k in range(NT):
                            nc.tensor.matmul(
                                ps[:, 0:GD], dT[:, k, 128 * m:128 * (m + 1)],
                                wvt[:, k, :],
                                start=(k == 0),
                                stop=(k == NT - 1 and not add_bias))
                        if add_bias:
                            nc.tensor.matmul(
                                ps[:, 0:GD], ones_row[:, 128 * m:128 * (m + 1)],
                                bv_sb[:], start=False, stop=True)
                        # scatter into [Vc|Vs|ones] layout (bf16 cast)
                        nc.any.tensor_copy(vall[:, m, :, 0:64], ps[:, 0:GD])

            # ---------------- Phase C: scratch projections ----------------
            with tc.tile_pool(name="phC", bufs=1) as pC, \
                 tc.tile_pool(name="phC_ps", bufs=2, space="PSUM") as pCp:
                sctT = pC.tile([64, NHL, T], F32R)
                wqs_sb = pC.tile([64, NHL, DS], F32R)
                wks_sb = pC.tile([64, NHL, DS], F32R)
                wvs_sb = pC.tile([64, NHL, DS], F32R)
                ksnat = pC.tile([128, NT, GD], F32R)
                for h in range(NHL):
                    nc.sync.dma_start(sctT[:, h, :], sct[h].bitcast(F32R))
                    nc.sync.dma_start(wqs_sb[:, h, :], wqs[h].bitcast(F32R))
                    nc.sync.dma_start(wks_sb[:, h, :], wks[h].bitcast(F32R))
                    nc.sync.dma_start(wvs_sb[:, h, :], wvs[h].bitcast(F32R))

                # Qs^T / Ks^T (base-0 psum; odd heads shifted via sbuf DMA)
                for h in range(NHL):
                    for (dst, w_sb, nm) in ((qsT2, wqs_sb, "q"), (ksT2, wks_sb, "k")):
                        ps = pCp.tile([64, T], F32, tag="scrT", name=f"ps_sc{nm}")
                        for n in range(NKC):
                            nc.tensor.matmul(
                                ps[:, 512 * n:512 * (n + 1)],
                                w_sb[:, h, :],
                                sctT[:, h, 512 * n:512 * (n + 1)],
                                start=True, stop=True)
                        if h % 2 == 0:
                            nc.any.tensor_copy(dst[0:64, h // 2, :], ps[:])
                        else:
                            tmpT = pC.tile([64, T], F32R, tag="tmpT", bufs=2)
                            nc.any.tensor_copy(tmpT[:], ps[:])
                            nc.sync.dma_start(dst[64:128, h // 2, :], tmpT[:])

                # Qs/Ks/Vs natural
                for m in range(NT):
                    psq = pCp.tile([128, GD], F32, tag="scrnat", name="ps_qs")
                    psk = pCp.tile([128, GD], F32, tag="scrnat", name="ps_ks")
                    psv = pCp.tile([128, GD], F32, tag="scrnat", name="ps_vs")
                    for h in range(NHL):
                        lhsT = sctT[:, h, 128 * m:128 * (m + 1)]
                        nc.tensor.matmul(psq[:, 64 * h:64 * h + 64], lhsT,
                                         wqs_sb[:, h, :], start=True, stop=True)
                        nc.tensor.matmul(psk[:, 64 * h:64 * h + 64], lhsT,
                                         wks_sb[:, h, :], start=True, stop=True)
                        nc.tensor.matmul(psv[:, 64 * h:64 * h + 64], lhsT,
                                         wvs_sb[:, h, :], start=True, stop=True)
                    nc.any.tensor_copy(qsnat[:, m, :], psq[:])
                    nc.any.tensor_copy(ksnat[:, m, :], psk[:])
                    nc.any.tensor_copy(vall[:, m, :, 64:128], psv[:])

                # ones column for the PV denominator
                nc.vector.memset(vall[:, :, :, 128:129], 1.0)

                # M_Ks per head (base-0 psum; odd heads shifted via sbuf DMA)
                for h in range(NHL):
                    mk = pCp.tile([128, 64], F32, tag="mksps", name="mk_s")
                    for m in range(NT):
                        nc.tensor.matmul(
                            mk[0:64, :],
                            ksnat[:, m, 64 * h:64 * h + 64],
                            ksnat[:, m, 64 * h:64 * h + 64],
                            start=(m == 0), stop=(m == NT - 1))
                    if h % 2 == 0:
                        nc.any.tensor_copy(gs_rhs[0:64, h // 2, 0:64], mk[0:64, :])
                    else:
                        mk_sb2 = pC.tile([64, 64], F32R, tag="mk_sb2", bufs=2)
                        nc.any.tensor_copy(mk_sb2[:], mk[0:64, :])
                        nc.sync.dma_start(gs_rhs[64:128, h // 2, 0:64], mk_sb2[:])

                # kcol_s
                with nc.allow_low_precision("f32r kcol sums, eps ~1e-3 ok"):
                    for h in range(NHL):
                        pb = (h % 2) * 64
                        nc.vector.tensor_reduce(
                            out=gs_rhs[pb:pb + 64, h // 2, 64:65],
                            in_=ksT2[pb:pb + 64, h // 2, :],
                            axis=mybir.AxisListType.X, op=mybir.AluOpType.add)

            # ---------------- Phase D: per-head attention ----------------
            with tc.tile_pool(name="phD", bufs=2) as pD, \
                 tc.tile_pool(name="phD_pT", bufs=2) as pDp, \
                 tc.tile_pool(name="psD_s", bufs=2, space="PSUM") as pDs, \
                 tc.tile_pool(name="psD_g", bufs=2, space="PSUM") as pDg, \
                 tc.tile_pool(name="psD_pv", bufs=2, space="PSUM") as pDpv:
                for h in range(NHL):
                    pb = (h % 2) * 64
                    pr = h // 2
                    pT = pDp.tile([128, NT, T], BF16, tag="pT")

                    # scores (transposed) + exp + diagonal mask
                    for j in range(NT):
                        ps = pDs.tile([128, T], F32, tag="score")
                        n0 = 0 if j < 4 else 1
                        for n in range(n0, NKC):
                            sl = slice(512 * n, 512 * (n + 1))
                            nc.tensor.matmul(
                                ps[:, sl],
                                (qkT[pb:pb + 64, 4 + pr, 128 * j:128 * (j + 1)]),
                                (qkT[pb:pb + 64, pr, sl]),
                                start=True, stop=not add_s)
                            if add_s:
                                nc.tensor.matmul(
                                    ps[:, sl],
                                    (ksT2[pb:pb + 64, pr, 128 * j:128 * (j + 1)]),
                                    (qsT2[pb:pb + 64, pr, sl]),
                                    start=False, stop=True)
                            nc.scalar.activation(
                                pT[:, j, sl], ps[:, sl],
                                mybir.ActivationFunctionType.Exp)
                        nd = j // 4
                        # zero P^T where q < k (diagonal chunk only)
                        nc.gpsimd.affine_select(
                            out=pT[:, j, 512 * nd:512 * (nd + 1)],
                            in_=pT[:, j, 512 * nd:512 * (nd + 1)],
                            compare_op=mybir.AluOpType.is_ge,
                            fill=0.0, base=512 * nd - 128 * j,
                            pattern=[[1, 512]], channel_multiplier=-1)

                    # G matmuls + row stats of unmasked logits
                    for i in range(NT):
                        gps = pDg.tile([128, 132], F32, tag="g")
                        nc.tensor.matmul(
                            gps[:, 0:66],
                            (qkT[pb:pb + 64, pr, 128 * i:128 * (i + 1)]),
                            (gc_rhs[pb:pb + 64, pr, 0:66]),
                            start=True, stop=True)
                        nc.tensor.matmul(
                            gps[:, 66:132],
                            (qsT2[pb:pb + 64, pr, 128 * i:128 * (i + 1)]),
                            (gs_rhs[pb:pb + 64, pr, 0:66]),
                            start=True, stop=True)
                        tmp = pD.tile([128, 64], F32, tag="tmp")
                        nc.vector.scalar_tensor_tensor(
                            out=tmp[:], in0=qnat[:, i, 64 * h:64 * h + 64],
                            scalar=1.0, in1=gps[:, 0:64],
                            op0=mybir.AluOpType.mult, op1=mybir.AluOpType.mult,
                            accum_out=stats_sb[:, i, 4 * h + 1:4 * h + 2])
                        tmp2 = pD.tile([128, 64], F32, tag="tmp")
                        nc.vector.scalar_tensor_tensor(
                            out=tmp2[:], in0=qsnat[:, i, 64 * h:64 * h + 64],
                            scalar=1.0, in1=gps[:, 66:130],
                            op0=mybir.AluOpType.mult, op1=mybir.AluOpType.mult,
                            accum_out=stats_sb[:, i, 4 * h + 3:4 * h + 4])
                        nc.any.tensor_copy(stats_sb[:, i, 4 * h:4 * h + 1],
                                           gps[:, 64:65])
                        nc.any.tensor_copy(stats_sb[:, i, 4 * h + 2:4 * h + 3],
                                           gps[:, 130:131])

                    # P @ [Vc|Vs|ones], normalize, emit
                    for i in range(NT):
                        pv = pDpv.tile([128, 132], F32, tag="pv")
                        for j in range(i + 1):
                            nc.tensor.matmul(
                                pv[:, 0:129],
                                pT[:, j, 128 * i:128 * (i + 1)],
                                vall[:, j, h, 0:129],
                                start=(j == 0), stop=(j == i))
                        rec = pD.tile([128, 1], F32, tag="rec")
                        nc.vector.reciprocal(rec[:], pv[:, 128:129])
                        nc.vector.tensor_scalar_mul(
                            content[:, i, 64 * h:64 * h + 64], pv[:, 0:64], rec[:])
                        scrt = pD.tile([128, 64], F32, tag="scr")
                        nc.vector.tensor_scalar_mul(scrt[:], pv[:, 64:128], rec[:])
                        nc.sync.dma_start(scro[h, 128 * i:128 * (i + 1), :], scrt[:])

            # ---------------- Phase E: output projection ----------------
            with tc.tile_pool(name="phE", bufs=1) as pE, \
                 tc.tile_pool(name="phE_o", bufs=2) as pEo, \
                 tc.tile_pool(name="psE_t", bufs=2, space="PSUM") as pEt, \
                 tc.tile_pool(name="psE_o", bufs=2, space="PSUM") as pEp:
                woT = pE.tile([128, 4, HD], F32R)
                for kk in range(4):
                    nc.sync.dma_start(woT[:, kk, :], wo[128 * kk:128 * (kk + 1), :].bitcast(F32R))
                contentT = pE.tile([128, 4, T], F32R)
                for kk in range(4):
                    for m in range(NT):
                        tps = pEt.tile([128, 128], F32, tag="tp")
                        nc.tensor.transpose(
                            tps[:], content[:, m, 128 * kk:128 * (kk + 1)], ident[:])
                        nc.any.tensor_copy(contentT[:, kk, 128 * m:128 * (m + 1)],
                                           tps[:])
                for m in range(NT):
                    ps = pEp.tile([128, HD], F32, tag="out")
                    for n in range(NKC):
                        for kk in range(4):
                            nc.tensor.matmul(
                                ps[:, 512 * n:512 * (n + 1)],
                                (contentT[:, kk, 128 * m:128 * (m + 1)]),
                                (woT[:, kk, 512 * n:512 * (n + 1)]),
                                start=(kk == 0), stop=(kk == 3))
                    osb = pEo.tile([128, HD], F32, tag="osb")
                    nc.any.tensor_copy(osb[:], ps[:])
                    nc.sync.dma_start(outp[128 * m:128 * (m + 1), :], osb[:])

                for i in range(NT):
                    nc.sync.dma_start(stats[i], stats_sb[:, i, :])

    nc.finalize()
    return nc


def kernel(data, scratch_in, qkv_w, qkv_b, out_w, out_b, Wq_s, Wk_s, Wv_s,
           lambda_s, per_head_scratch_val):
    data = np.asarray(data, np.float32)
    scratch_in = np.asarray(scratch_in, np.float32)
    qkv_w = np.asarray(qkv_w, np.float32)
    qkv_b = np.asarray(qkv_b, np.float32)
    out_w = np.asarray(out_w, np.float32)
    out_b = np.asarray(out_b, np.float32)
    Wq_s = np.asarray(Wq_s, np.float32)
    Wk_s = np.asarray(Wk_s, np.float32)
    Wv_s = np.asarray(Wv_s, np.float32)

    lam = float(np.asarray(lambda_s))
    gate = 1.0 if lam > 1e-8 else 0.0
    phv = np.exp(np.asarray(per_head_scratch_val, np.float64))      # [NH]
    c_h = gate * lam * phv                                          # [NH]
    add_s = bool(np.any(c_h > 0))
    add_bias = bool(np.any(qkv_b != 0))

    key = (add_s, add_bias)
    if key not in _prog_cache:
        _prog_cache[key] = _build_program(add_s, add_bias)
    nc = _prog_cache[key]

    sdh = 1.0 / np.sqrt(np.float64(DH))
    sds = 1.0 / np.sqrt(np.float64(DS))

    in_maps = []
    for core in range(N_CORES):
        b, g = divmod(core, 2)
        H = slice(NHL * g, NHL * (g + 1))
        rows = slice(GD * g, GD * (g + 1))
        wq = qkv_w[0 * HD:1 * HD][rows] * np.float32(sdh)
        wk = qkv_w[1 * HD:2 * HD][rows]
        m = {
            "data_t": np.ascontiguousarray(data[b].T),
            "wqk_t": np.ascontiguousarray(np.concatenate([wq, wk], 0).T),
            "wv_t": np.ascontiguousarray(qkv_w[2 * HD:3 * HD][rows].T),
            "scratch_t": np.ascontiguousarray(scratch_in[b, H].transpose(0, 2, 1)),
            "wqs": np.ascontiguousarray(
                Wq_s[H] * (np.where(c_h[H] > 0, c_h[H], 1.0) * sds
                           )[:, None, None].astype(np.float32)),
            "wks": np.ascontiguousarray(Wk_s[H]),
            "wvs": np.ascontiguousarray(Wv_s[H]),
            "wo_t": np.ascontiguousarray(out_w[:, rows].T),
        }
        if add_bias:
            bq = qkv_b[0 * HD:1 * HD][rows] * np.float32(sdh)
            bk = qkv_b[1 * HD:2 * HD][rows]
            m["bqk"] = np.concatenate([bq, bk])[None, :].astype(np.float32)
            m["bv"] = qkv_b[2 * HD:3 * HD][rows][None, :].astype(np.float32)
        in_maps.append(m)

    trace = bool(LAST.get("trace"))
    res = run_bass_kernel_spmd(nc, in_maps, core_ids=list(range(N_CORES)),
                               trace=trace)
    LAST["exec_time_ns"] = res.exec_time_ns

    attn = np.zeros((B, T, HD), np.float64)
    scr = np.empty((B, NH, T, DS), np.float32)
    # per (b,h,q): sum_c, sumsq_c, sum_s(hat), sumsq_s(hat)
    st = np.empty((B, NH, T, 4), np.float64)
    for core in range(N_CORES):
        b, g = divmod(core, 2)
        H = slice(NHL * g, NHL * (g + 1))
        r = res.results[core]
        attn[b] += r["partial"].astype(np.float64)
        scr[b, H] = r["scr_out"]
        # stats dram [NT, 128, 4*NHL] -> [T, NHL, 4]
        s = r["stats"].reshape(T, NHL, 4).astype(np.float64)
        st[b, H] = s.transpose(1, 0, 2)

    attn += out_b.astype(np.float64)
    attn_out = attn.astype(np.float32)

    # scalars (float64 host math)
    sum_c, sumsq_c = st[..., 0], st[..., 1]
    sum_s, sumsq_s = st[..., 2], st[..., 3]
    n = np.float64(T)
    var_c = np.maximum(sumsq_c - sum_c ** 2 / n, 0.0) / (n - 1.0)
    var_s = np.maximum(sumsq_s - sum_s ** 2 / n, 0.0) / (n - 1.0)
    std_c = np.sqrt(var_c)
    std_s_hat = np.sqrt(var_s)                       # std of s-hat logits
    # s-hat = c_h * logits_s when gated else logits_s
    scale_back = np.where(c_h > 0, c_h, 1.0)[None, :, None]
    std_s = std_s_hat / scale_back
    row_std_ratio = np.float32(np.mean(std_s / (std_c + 1e-6)))
    if add_s:
        delta_rms = np.float32(np.sqrt(sumsq_s.sum() / (B * NH * T * T)))
    else:
        delta_rms = np.float32(0.0)

    return attn_out, scr, delta_rms, row_std_ratio
